# revision 15
# baseline (speedup 1.0000x reference)
"""Trainium2 Bass kernel for nn_AudioVisualModel audio-visual contrastive loss.

Strategy (8 NeuronCores, SPMD):
  - Shard the visual batch axis: core m owns y in {2m, 2m+1}. Every core gets
    the full (normalized, transposed) audio features (2 MB) plus its own 4 MB
    visual shard, so total HBM traffic is ~6 MB/core instead of ~32 MB/core
    for audio-axis sharding.
  - Host: L2-normalize both inputs (fp32), lay audio out as AT[k][128d, 2048tok]
    and visual as VT[k][128d, 3920] in v-major order (col = yl*1960 + v*10 + t)
    so that each PSUM bank chunk of 490 columns covers 49 full v-groups.
  - Device, per (x, yl) slab (32 slabs): 8 fp32r matmuls (K=2x128, N=490) into
    a 4-bank PSUM slab; one DVE tensor_reduce(axis=XY) produces max-over-v
    [128 tokens, 10 t]; the non-negativity term sum(min(s,0)^2) runs on
    ScalarE (Relu(-s) then Square with accum_out) for 3 of 4 slabs and on
    VectorE (scalar_tensor_tensor min/mult with accum_out) for the rest to
    balance engine load.
  - Output per core: [128, 352] = 320 cols of per-(slab,t) max values plus 32
    per-slab nonneg partials; host does the partition sums, the 16x16 InfoNCE
    softmax, and temperature scaling (max/mean/min-square all commute with the
    positive temperature divide, so the device works on raw cosines).
"""
import sys

sys.path.insert(0, "/opt/trn_rl_repo")

import numpy as np

B, NA, T, NV, D = 16, 128, 10, 196, 256
N_CORES = 8
Y_PER_CORE = B // N_CORES          # 2
COLS_PER_Y = T * NV                # 1960
N_SLABS = B * Y_PER_CORE           # 32 per core
BANKW = 512                        # fp32 psum bank width
CHUNK = 490                        # 49 v-groups * 10 t per bank
NBANK = 4                          # banks per slab (4*490 = 1960)
VG = 49                            # v-groups per bank chunk
# Pass-B assignment: P2 slabs do ACT relu + DVE square-sum (hybrid); the rest
# (P1) do both relu and square on ACT. Balances DVE (max-reduce heavy) vs ACT.
N_HYBRID = 13
HYBRID_SLABS = frozenset(
    round(i * N_SLABS / N_HYBRID) for i in range(N_HYBRID))
N_STT = len(HYBRID_SLABS)
N_ACT = N_SLABS - N_STT
OUT_COLS = N_SLABS * T + N_SLABS   # 320 + 32 = 352

_PROG_CACHE = {}


def _build_program(mm_dtype_name="float32r", loop_reps=1):
    """loop_reps > 1 wraps the compute pass in a hardware loop (used only by
    the timing harness to measure per-iteration HW time differentially)."""
    import contextlib

    import concourse.tile as tile
    from concourse import bacc, mybir

    mm_dt = getattr(mybir.dt, mm_dtype_name)
    f32 = mybir.dt.float32

    nc = bacc.Bacc("TRN2", target_bir_lowering=False, debug=False,
                   num_devices=N_CORES)
    at_d = nc.declare_dram_parameter("at", [2, 128, 2048], mm_dt, isOutput=False)
    vt_d = nc.declare_dram_parameter("vt", [2, 128, 2 * COLS_PER_Y], mm_dt,
                                     isOutput=False)
    out_d = nc.declare_dram_parameter("out", [128, OUT_COLS], f32, isOutput=True)

    with tile.TileContext(nc) as tc:
        with (
            tc.tile_pool(name="persist", bufs=1) as pp,
            tc.tile_pool(name="scratch", bufs=2) as zp,
            tc.tile_pool(name="psum", bufs=2, space="PSUM") as ps,
        ):
            # Persistent input tiles, chunked so DMA deps stay fine-grained.
            at_t = [[pp.tile([128, 512], mm_dt, name=f"at{k}_{g}",
                             tag=f"at{k}_{g}") for g in range(4)]
                    for k in range(2)]
            vt_t = [[[pp.tile([128, CHUNK], mm_dt, name=f"vt{k}_{yl}_{b}",
                              tag=f"vt{k}_{yl}_{b}") for b in range(NBANK)]
                     for yl in range(2)]
                    for k in range(2)]
            tm = pp.tile([128, N_SLABS * T], f32, name="tm", tag="tm")
            nn_dve = pp.tile([128, N_STT], f32, name="nn_dve", tag="nn_dve")
            nn_act = pp.tile([128, N_ACT], f32, name="nn_act", tag="nn_act")
            dummy = pp.tile([128, 1], f32, name="dummy", tag="dummy")

            # Tiny activation issued first so the ACT function-table load
            # happens during the DMA lead-in, off the critical path.
            nc.vector.memset(dummy[:], 0.0)
            nc.scalar.activation(out=dummy[:], in_=dummy[:],
                                 func=mybir.ActivationFunctionType.Relu)
            nc.scalar.activation(out=dummy[:], in_=dummy[:],
                                 func=mybir.ActivationFunctionType.Square)

            # DMA issue order tracks first use: the first slab (yl=0, x=0)
            # needs at[.][0] and all 4 vt[.][0] bank chunks; later audio
            # groups and the yl=1 shard stream in behind.
            nc.sync.dma_start(at_t[0][0][:], at_d[0, :, 0:512])
            nc.sync.dma_start(at_t[1][0][:], at_d[1, :, 0:512])
            for b in range(NBANK):
                for k in range(2):
                    nc.sync.dma_start(
                        vt_t[k][0][b][:],
                        vt_d[k, :, b * CHUNK:(b + 1) * CHUNK])
            for g in range(1, 4):
                for k in range(2):
                    nc.sync.dma_start(
                        at_t[k][g][:], at_d[k, :, g * 512:(g + 1) * 512])
            for b in range(NBANK):
                for k in range(2):
                    nc.sync.dma_start(
                        vt_t[k][1][b][:],
                        vt_d[k, :, COLS_PER_Y + b * CHUNK:
                             COLS_PER_Y + (b + 1) * CHUNK])

            if loop_reps > 1:
                loop_cm = tc.For_i(0, loop_reps, 1,
                                   hint_engines=(mybir.EngineType.PE,))
            else:
                loop_cm = contextlib.nullcontext()
            loop_stack = contextlib.ExitStack()
            loop_stack.enter_context(loop_cm)
            n_act_seen = 0
            n_stt_seen = 0
            for i in range(N_SLABS):
                yl, x = divmod(i, B)
                slab = ps.tile([128, NBANK * BANKW], f32, name=f"slab{i}",
                               tag="slab")
                for k in range(2):
                    lhsT = at_t[k][x // 4][:, (x % 4) * 128:(x % 4 + 1) * 128]
                    for b in range(NBANK):
                        nc.tensor.matmul(
                            slab[:, b * BANKW:b * BANKW + CHUNK],
                            lhsT=lhsT,
                            rhs=vt_t[k][yl][b][:, 0:CHUNK],
                            start=(k == 0), stop=(k == 1))

                # [128, bank, 490] view of the live columns
                banks = slab[:].rearrange("p (b c) -> p b c", b=NBANK)[:, :, 0:CHUNK]
                # [128, t, bank, j]: max over (bank, j) = max over all 196 v
                red_in = banks.rearrange("p b (j t) -> p t b j", t=T)
                if i != 0:
                    # Same-bank PSUM reads serialize across engines; in steady
                    # state reduce-then-relu is fine, but for the first slab
                    # the relu goes first so ScalarE (the busiest engine)
                    # starts as early as possible (see the i == 0 case below).
                    nc.vector.tensor_reduce(
                        out=tm[:, i * T:(i + 1) * T], in_=red_in,
                        axis=mybir.AxisListType.XY, op=mybir.AluOpType.max)

                if i in HYBRID_SLABS:
                    # hybrid: ScalarE computes z = relu(-s) (bf16, SBUF);
                    # VectorE squares+sums it (both operands SBUF -> legal).
                    zb = zp.tile([128, NBANK * CHUNK], mybir.dt.bfloat16,
                                 name=f"zb_{i}", tag="zb")
                    zbv = zb[:].rearrange("p (b c) -> p b c", b=NBANK)
                    nc.scalar.activation(
                        out=zbv, in_=banks,
                        func=mybir.ActivationFunctionType.Relu, scale=-1.0)
                    s2 = zp.tile([128, NBANK * CHUNK], mybir.dt.bfloat16,
                                 name=f"s2_{i}", tag="s2")
                    j = n_stt_seen
                    n_stt_seen += 1
                    nc.vector.scalar_tensor_tensor(
                        out=s2[:], in0=zb[:], scalar=0.0, in1=zb[:],
                        op0=mybir.AluOpType.bypass, op1=mybir.AluOpType.mult,
                        accum_out=nn_dve[:, j:j + 1])
                else:
                    z = zp.tile([128, NBANK * CHUNK], f32, name=f"z_{i}",
                                tag="z")
                    zv = z[:].rearrange("p (b c) -> p b c", b=NBANK)
                    nc.scalar.activation(
                        out=zv, in_=banks,
                        func=mybir.ActivationFunctionType.Relu, scale=-1.0)
                    nc.scalar.activation(
                        out=z[:], in_=z[:],
                        func=mybir.ActivationFunctionType.Square,
                        accum_out=nn_act[:, n_act_seen:n_act_seen + 1])
                    n_act_seen += 1

                if i == 0:
                    nc.vector.tensor_reduce(
                        out=tm[:, i * T:(i + 1) * T], in_=red_in,
                        axis=mybir.AxisListType.XY, op=mybir.AluOpType.max)

            loop_stack.close()
            nc.sync.dma_start(out_d[:, 0:N_SLABS * T], tm[:])
            nc.sync.dma_start(
                out_d[:, N_SLABS * T:N_SLABS * T + N_STT], nn_dve[:])
            nc.sync.dma_start(out_d[:, N_SLABS * T + N_STT:OUT_COLS], nn_act[:])

    nc.compile()
    return nc


def _get_program(mm_dtype_name="float32r", loop_reps=1):
    key = (mm_dtype_name, loop_reps)
    if key not in _PROG_CACHE:
        _PROG_CACHE[key] = _build_program(mm_dtype_name, loop_reps)
    return _PROG_CACHE[key]


def _prep_inputs(audio_feats, visual_feats):
    a = np.ascontiguousarray(np.asarray(audio_feats, dtype=np.float32))
    v = np.ascontiguousarray(np.asarray(visual_feats, dtype=np.float32))
    an = a / np.maximum(
        np.sqrt((a * a).sum(-1, keepdims=True, dtype=np.float32)), 1e-12)
    vn = v / np.maximum(
        np.sqrt((v * v).sum(-1, keepdims=True, dtype=np.float32)), 1e-12)

    # AT[k, d, tok]; tok = x*128 + a_tok, d split as k*128 + dd (d-major)
    at = np.ascontiguousarray(
        an.reshape(B * NA, 2, 128).transpose(1, 2, 0))
    in_maps = []
    for m in range(N_CORES):
        vloc = vn[2 * m:2 * m + 2]                      # (2, T, NV, D)
        vt = vloc.reshape(2, T, NV, 2, 128).transpose(3, 4, 0, 2, 1)
        vt = np.ascontiguousarray(vt).reshape(2, 128, 2 * COLS_PER_Y)
        in_maps.append({"at": at, "vt": vt})
    return in_maps


def _finalize(core_outs, temperature):
    """core_outs: list of 8 arrays [128, 352] (fp32). Host-side gather."""
    Tf = float(temperature)
    clip = np.zeros((B, B), dtype=np.float64)
    nonneg_sum = 0.0
    for m, out in enumerate(core_outs):
        colsum = out.astype(np.float64).sum(axis=0)      # [352]
        tmsum = colsum[:N_SLABS * T].reshape(2, B, T)    # [yl, x, t]
        clip[:, 2 * m] = tmsum[0].sum(axis=1)
        clip[:, 2 * m + 1] = tmsum[1].sum(axis=1)
        nonneg_sum += colsum[N_SLABS * T:].sum()

    clip /= (NA * T)            # mean over audio tokens and time
    clip /= Tf                  # temperature (commutes with max/mean)

    # InfoNCE on the diagonal
    def log_softmax_diag(mat):
        mx = mat.max(axis=1, keepdims=True)
        lse = np.log(np.exp(mat - mx).sum(axis=1)) + mx[:, 0]
        return np.diag(mat) - lse

    losses = -(log_softmax_diag(clip) + log_softmax_diag(clip.T))
    contrastive = 0.5 * losses.mean()

    l_nonneg = nonneg_sum / (B * B * NA * T * NV) / (Tf * Tf)
    log_t = np.log(Tf)
    temp_low = max(-log_t, 0.0) ** 4
    temp_high = max(log_t - np.log(3.0), 0.0) ** 4
    reg = l_nonneg + temp_low + temp_high
    total = contrastive + 0.3 * reg
    return (np.float32(total), np.float32(contrastive), np.float32(reg))


def kernel(audio_feats, visual_feats, temperature):
    from concourse.bass_utils import run_bass_kernel_spmd

    nc = _get_program()
    in_maps = _prep_inputs(audio_feats, visual_feats)
    res = run_bass_kernel_spmd(nc, in_maps, list(range(N_CORES)))
    core_outs = [res.results[m]["out"] for m in range(N_CORES)]
    return _finalize(core_outs, temperature)


# revision 17
# speedup vs baseline: 1.4283x; 1.4283x over previous
"""Trainium2 Bass kernel for nn_AudioVisualModel audio-visual contrastive loss.

Strategy (8 NeuronCores, SPMD):
  - Shard the visual batch axis: core m owns y in {2m, 2m+1}. Every core gets
    the full (normalized, transposed) audio features (2 MB) plus its own 4 MB
    visual shard, so total HBM traffic is ~6 MB/core instead of ~32 MB/core
    for audio-axis sharding.
  - Host: L2-normalize both inputs (fp32), lay audio out as AT[k][128d, 2048tok]
    and visual as VT[k][128d, 3920] in v-major order (col = yl*1960 + v*10 + t)
    so that each PSUM bank chunk of 490 columns covers 49 full v-groups.
  - Device, per (x, yl) slab (32 slabs): 8 fp32r matmuls (K=2x128, N=490) into
    a 4-bank PSUM slab; one DVE tensor_reduce(axis=XY) produces max-over-v
    [128 tokens, 10 t]; the non-negativity term sum(min(s,0)^2) runs on
    ScalarE (Relu(-s) then Square with accum_out) for 3 of 4 slabs and on
    VectorE (scalar_tensor_tensor min/mult with accum_out) for the rest to
    balance engine load.
  - Output per core: [128, 352] = 320 cols of per-(slab,t) max values plus 32
    per-slab nonneg partials; host does the partition sums, the 16x16 InfoNCE
    softmax, and temperature scaling (max/mean/min-square all commute with the
    positive temperature divide, so the device works on raw cosines).
"""
import sys

sys.path.insert(0, "/opt/trn_rl_repo")

import numpy as np

B, NA, T, NV, D = 16, 128, 10, 196, 256
N_CORES = 8
Y_PER_CORE = B // N_CORES          # 2
COLS_PER_Y = T * NV                # 1960
N_SLABS = B * Y_PER_CORE           # 32 per core
BANKW = 512                        # fp32 psum bank width
CHUNK = 490                        # 49 v-groups * 10 t per bank
NBANK = 4                          # banks per slab (4*490 = 1960)
VG = 49                            # v-groups per bank chunk
# Pass-B assignment: P2 slabs do ACT relu + DVE square-sum (hybrid); the rest
# (P1) do both relu and square on ACT. Balances DVE (max-reduce heavy) vs ACT.
N_HYBRID = 13
HYBRID_SLABS = frozenset(
    round(i * N_SLABS / N_HYBRID) for i in range(N_HYBRID))
N_STT = len(HYBRID_SLABS)
N_ACT = N_SLABS - N_STT
OUT_COLS = N_SLABS * T + N_SLABS   # 320 + 32 = 352

_PROG_CACHE = {}


def _build_program(mm_dtype_name="float32r", loop_reps=1, variant="full"):
    """loop_reps > 1 wraps the compute pass in a hardware loop (used only by
    the timing harness to measure per-iteration HW time differentially).
    variant: full | nopassb | nomax | mmonly (stripped builds for profiling)."""
    import contextlib

    import concourse.tile as tile
    from concourse import bacc, mybir

    do_max = variant in ("full", "nopassb")
    do_passb = variant in ("full", "nomax")
    mm_dt = getattr(mybir.dt, mm_dtype_name)
    f32 = mybir.dt.float32

    nc = bacc.Bacc("TRN2", target_bir_lowering=False, debug=False,
                   num_devices=N_CORES)
    at_d = nc.declare_dram_parameter("at", [2, 128, 2048], mm_dt, isOutput=False)
    vt_d = nc.declare_dram_parameter("vt", [2, 128, 2 * COLS_PER_Y], mm_dt,
                                     isOutput=False)
    out_d = nc.declare_dram_parameter("out", [128, OUT_COLS], f32, isOutput=True)

    with tile.TileContext(nc) as tc:
        with (
            tc.tile_pool(name="persist", bufs=1) as pp,
            tc.tile_pool(name="scratch", bufs=2) as zp,
            tc.tile_pool(name="psum", bufs=2, space="PSUM") as ps,
        ):
            # Persistent input tiles, chunked so DMA deps stay fine-grained.
            at_t = [[pp.tile([128, 512], mm_dt, name=f"at{k}_{g}",
                             tag=f"at{k}_{g}") for g in range(4)]
                    for k in range(2)]
            vt_t = [[[pp.tile([128, CHUNK], mm_dt, name=f"vt{k}_{yl}_{b}",
                              tag=f"vt{k}_{yl}_{b}") for b in range(NBANK)]
                     for yl in range(2)]
                    for k in range(2)]
            tm = pp.tile([128, N_SLABS * T], f32, name="tm", tag="tm")
            nn_dve = pp.tile([128, N_STT], f32, name="nn_dve", tag="nn_dve")
            nn_act = pp.tile([128, N_ACT], f32, name="nn_act", tag="nn_act")
            dummy = pp.tile([128, 1], f32, name="dummy", tag="dummy")

            # Tiny activation issued first so the ACT function-table load
            # happens during the DMA lead-in, off the critical path.
            nc.vector.memset(dummy[:], 0.0)
            nc.scalar.activation(out=dummy[:], in_=dummy[:],
                                 func=mybir.ActivationFunctionType.Relu)
            nc.scalar.activation(out=dummy[:], in_=dummy[:],
                                 func=mybir.ActivationFunctionType.Square)

            # DMA issue order tracks first use: the first slab (yl=0, x=0)
            # needs at[.][0] and all 4 vt[.][0] bank chunks; later audio
            # groups and the yl=1 shard stream in behind.
            nc.sync.dma_start(at_t[0][0][:], at_d[0, :, 0:512])
            nc.sync.dma_start(at_t[1][0][:], at_d[1, :, 0:512])
            for b in range(NBANK):
                for k in range(2):
                    nc.sync.dma_start(
                        vt_t[k][0][b][:],
                        vt_d[k, :, b * CHUNK:(b + 1) * CHUNK])
            for g in range(1, 4):
                for k in range(2):
                    nc.sync.dma_start(
                        at_t[k][g][:], at_d[k, :, g * 512:(g + 1) * 512])
            for b in range(NBANK):
                for k in range(2):
                    nc.sync.dma_start(
                        vt_t[k][1][b][:],
                        vt_d[k, :, COLS_PER_Y + b * CHUNK:
                             COLS_PER_Y + (b + 1) * CHUNK])

            if loop_reps > 1:
                loop_cm = tc.For_i(0, loop_reps, 1,
                                   hint_engines=(mybir.EngineType.PE,))
            else:
                loop_cm = contextlib.nullcontext()
            loop_stack = contextlib.ExitStack()
            loop_stack.enter_context(loop_cm)
            n_act_seen = 0
            n_stt_seen = 0
            for i in range(N_SLABS):
                yl, x = divmod(i, B)
                slab = ps.tile([128, NBANK * BANKW], f32, name=f"slab{i}",
                               tag="slab")
                for k in range(2):
                    lhsT = at_t[k][x // 4][:, (x % 4) * 128:(x % 4 + 1) * 128]
                    for b in range(NBANK):
                        nc.tensor.matmul(
                            slab[:, b * BANKW:b * BANKW + CHUNK],
                            lhsT=lhsT,
                            rhs=vt_t[k][yl][b][:, 0:CHUNK],
                            start=(k == 0), stop=(k == 1))

                # [128, bank, 490] view of the live columns
                banks = slab[:].rearrange("p (b c) -> p b c", b=NBANK)[:, :, 0:CHUNK]
                # [128, t, bank, j]: max over (bank, j) = max over all 196 v
                red_in = banks.rearrange("p b (j t) -> p t b j", t=T)
                if i != 0 and do_max:
                    # Same-bank PSUM reads serialize across engines; in steady
                    # state reduce-then-relu is fine, but for the first slab
                    # the relu goes first so ScalarE (the busiest engine)
                    # starts as early as possible (see the i == 0 case below).
                    nc.vector.tensor_reduce(
                        out=tm[:, i * T:(i + 1) * T], in_=red_in,
                        axis=mybir.AxisListType.XY, op=mybir.AluOpType.max)

                if not do_passb:
                    n_act_seen += 0
                elif i in HYBRID_SLABS:
                    # hybrid: ScalarE computes z = relu(-s) (bf16, SBUF);
                    # VectorE squares+sums it (both operands SBUF -> legal).
                    zb = zp.tile([128, NBANK * CHUNK], mybir.dt.bfloat16,
                                 name=f"zb_{i}", tag="zb")
                    zbv = zb[:].rearrange("p (b c) -> p b c", b=NBANK)
                    nc.scalar.activation(
                        out=zbv, in_=banks,
                        func=mybir.ActivationFunctionType.Relu, scale=-1.0)
                    s2 = zp.tile([128, NBANK * CHUNK], mybir.dt.bfloat16,
                                 name=f"s2_{i}", tag="s2")
                    j = n_stt_seen
                    n_stt_seen += 1
                    nc.vector.scalar_tensor_tensor(
                        out=s2[:], in0=zb[:], scalar=0.0, in1=zb[:],
                        op0=mybir.AluOpType.bypass, op1=mybir.AluOpType.mult,
                        accum_out=nn_dve[:, j:j + 1])
                else:
                    z = zp.tile([128, NBANK * CHUNK], f32, name=f"z_{i}",
                                tag="z")
                    zv = z[:].rearrange("p (b c) -> p b c", b=NBANK)
                    nc.scalar.activation(
                        out=zv, in_=banks,
                        func=mybir.ActivationFunctionType.Relu, scale=-1.0)
                    nc.scalar.activation(
                        out=z[:], in_=z[:],
                        func=mybir.ActivationFunctionType.Square,
                        accum_out=nn_act[:, n_act_seen:n_act_seen + 1])
                    n_act_seen += 1

                if i == 0 and do_max:
                    nc.vector.tensor_reduce(
                        out=tm[:, i * T:(i + 1) * T], in_=red_in,
                        axis=mybir.AxisListType.XY, op=mybir.AluOpType.max)

            loop_stack.close()
            if do_max:
                nc.sync.dma_start(out_d[:, 0:N_SLABS * T], tm[:])
            if do_passb:
                nc.sync.dma_start(
                    out_d[:, N_SLABS * T:N_SLABS * T + N_STT], nn_dve[:])
                nc.sync.dma_start(
                    out_d[:, N_SLABS * T + N_STT:OUT_COLS], nn_act[:])

    nc.compile()
    return nc


def _get_program(mm_dtype_name="float32r", loop_reps=1):
    key = (mm_dtype_name, loop_reps)
    if key not in _PROG_CACHE:
        _PROG_CACHE[key] = _build_program(mm_dtype_name, loop_reps)
    return _PROG_CACHE[key]


def _prep_inputs(audio_feats, visual_feats):
    a = np.ascontiguousarray(np.asarray(audio_feats, dtype=np.float32))
    v = np.ascontiguousarray(np.asarray(visual_feats, dtype=np.float32))
    an = a / np.maximum(
        np.sqrt((a * a).sum(-1, keepdims=True, dtype=np.float32)), 1e-12)
    vn = v / np.maximum(
        np.sqrt((v * v).sum(-1, keepdims=True, dtype=np.float32)), 1e-12)

    # AT[k, d, tok]; tok = x*128 + a_tok, d split as k*128 + dd (d-major)
    at = np.ascontiguousarray(
        an.reshape(B * NA, 2, 128).transpose(1, 2, 0))
    in_maps = []
    for m in range(N_CORES):
        vloc = vn[2 * m:2 * m + 2]                      # (2, T, NV, D)
        vt = vloc.reshape(2, T, NV, 2, 128).transpose(3, 4, 0, 2, 1)
        vt = np.ascontiguousarray(vt).reshape(2, 128, 2 * COLS_PER_Y)
        in_maps.append({"at": at, "vt": vt})
    return in_maps


def _finalize(core_outs, temperature):
    """core_outs: list of 8 arrays [128, 352] (fp32). Host-side gather."""
    Tf = float(temperature)
    clip = np.zeros((B, B), dtype=np.float64)
    nonneg_sum = 0.0
    for m, out in enumerate(core_outs):
        colsum = out.astype(np.float64).sum(axis=0)      # [352]
        tmsum = colsum[:N_SLABS * T].reshape(2, B, T)    # [yl, x, t]
        clip[:, 2 * m] = tmsum[0].sum(axis=1)
        clip[:, 2 * m + 1] = tmsum[1].sum(axis=1)
        nonneg_sum += colsum[N_SLABS * T:].sum()

    clip /= (NA * T)            # mean over audio tokens and time
    clip /= Tf                  # temperature (commutes with max/mean)

    # InfoNCE on the diagonal
    def log_softmax_diag(mat):
        mx = mat.max(axis=1, keepdims=True)
        lse = np.log(np.exp(mat - mx).sum(axis=1)) + mx[:, 0]
        return np.diag(mat) - lse

    losses = -(log_softmax_diag(clip) + log_softmax_diag(clip.T))
    contrastive = 0.5 * losses.mean()

    l_nonneg = nonneg_sum / (B * B * NA * T * NV) / (Tf * Tf)
    log_t = np.log(Tf)
    temp_low = max(-log_t, 0.0) ** 4
    temp_high = max(log_t - np.log(3.0), 0.0) ** 4
    reg = l_nonneg + temp_low + temp_high
    total = contrastive + 0.3 * reg
    return (np.float32(total), np.float32(contrastive), np.float32(reg))


def kernel(audio_feats, visual_feats, temperature):
    from concourse.bass_utils import run_bass_kernel_spmd

    nc = _get_program()
    in_maps = _prep_inputs(audio_feats, visual_feats)
    res = run_bass_kernel_spmd(nc, in_maps, list(range(N_CORES)))
    core_outs = [res.results[m]["out"] for m in range(N_CORES)]
    return _finalize(core_outs, temperature)
